# revision 11
# baseline (speedup 1.0000x reference)
"""GAT (graph attention) convolution on 8 Trainium2 NeuronCores.

Strategy (all-standard + validated primitives):
  - Destination-shard the edges: core c owns dst nodes [c*6272, (c+1)*6272).
    Each core's x input is ROTATED so its own nodes are table rows 0..6271
    (keeps all dma_gather int16 indices in range; gather tables are split
    at row 32768 for the signed-int16 limit).
  - Phase A (per core): h_ext = x @ [W | W@att_src_blk | W@att_dst_blk]
    -> DRAM table htab[rows, 128] = [h(64) | a_src(4) | a_dst(4) | junk].
  - Phase B: edges sorted by dst window (128 dst nodes per window).
    dma_gather h_ext rows by src (512B rows) and a-values by dst (256B).
    p = exp(leaky_relu(a_src[src] + a_dst[dst])); V = [p*h | p].
    Segment-sum into the window via one-hot matmuls accumulated in PSUM
    (one-hot built with tensor_scalar is_equal against an iota row).
  - Phase C: out = S/(D + 1e-16) + bias for the core's own 6272 nodes.

No max-subtraction in the softmax: e has tiny dynamic range (|e| < ~8), so
exp(e)/sum(exp(e)) matches the reference's exp(e-m)/sum(exp(e-m)) to fp32
rounding.
"""
import numpy as np

# ---------------------------------------------------------------- constants
N = 50000
E = 800000
IN = 128
H = 4
C = 16
HC = H * C            # 64
EXT = 72              # h(64) + a_src(4) + a_dst(4)
NEG_SLOPE = 0.2
NCORES = 8
NLOC = 6272           # 49*128 nodes per core
NW = NLOC // 128      # 49 windows per core
NPAD = NLOC * NCORES  # 50176
HTAB_ROWS = 50304     # 393*128 = NPAD + 128
SPLIT = 32768         # int16 gather index limit
CALL_BLOCKS = 8       # 8*128 = 1024 idxs per dma_gather call (SWDGE ring cap)

_cache = {}


def _build_host_data(edge_index):
    """Sort/shard/pad edges; returns per-core index/wrel arrays and caps."""
    src = edge_index[0].astype(np.int64)
    dst = edge_index[1].astype(np.int64)
    # self-loops as ordinary edges
    loops = np.arange(N, dtype=np.int64)
    src = np.concatenate([src, loops])
    dst = np.concatenate([dst, loops])

    core = dst // NLOC
    per_core = []
    perms = []
    capA = capB = 0
    for c in range(NCORES):
        m = core == c
        s_rot = (src[m] - c * NLOC) % NPAD
        d_loc = dst[m] - c * NLOC
        # balance windows: assign nodes to windows greedily by degree so every
        # window gets ~equal edge count (output rows are permuted; undone on
        # the host after the run)
        deg = np.bincount(d_loc, minlength=NLOC)
        order_n = np.argsort(-deg, kind="stable")
        wcnt = np.zeros(NW, np.int64)
        wfill = np.zeros(NW, np.int64)
        pos = np.zeros(NLOC, np.int64)     # node -> device row (w*128+wrel)
        for nd in order_n:
            cand = np.where(wfill < 128)[0]
            wi = cand[np.argmin(wcnt[cand])]
            pos[nd] = wi * 128 + wfill[wi]
            wcnt[wi] += deg[nd]
            wfill[wi] += 1
        perms.append(pos)
        w = pos[d_loc] >> 7
        grp = (s_rot >= SPLIT).astype(np.int64)
        order = np.lexsort((s_rot, grp, w))
        s_rot, d_loc, w, grp = s_rot[order], d_loc[order], w[order], grp[order]
        cntA = np.bincount(w[grp == 0], minlength=NW)
        cntB = np.bincount(w[grp == 1], minlength=NW)
        capA = max(capA, int(np.ceil(cntA.max() / 128)))
        capB = max(capB, int(np.ceil(cntB.max() / 128)))
        per_core.append((s_rot, d_loc, w, grp, cntA, cntB, pos))

    NB = capA + capB
    cores = []
    for c in range(NCORES):
        s_rot, d_loc, w, grp, cntA, cntB, pos = per_core[c]
        wrel_of = pos & 127
        srcidx = np.zeros((NW, NB, 128), np.int64)
        dstidx = np.zeros((NW, NB, 128), np.int64)
        wrelf = np.full((NW, NB, 128), 999.0, np.float32)
        # edges are sorted by (w, grp); compute start offset of each (w,grp)
        startA = np.concatenate([[0], np.cumsum(cntA + cntB)[:-1]])
        for wi in range(NW):
            a0 = startA[wi]
            nA = cntA[wi]
            nB = cntB[wi]
            segA = slice(a0, a0 + nA)
            segB = slice(a0 + nA, a0 + nA + nB)
            fa = srcidx[wi, :capA].reshape(-1)
            fa[:nA] = s_rot[segA]
            fb = srcidx[wi, capA:].reshape(-1)
            fb[:nB] = s_rot[segB] - SPLIT
            da = dstidx[wi, :capA].reshape(-1)
            da[:nA] = d_loc[segA]
            db = dstidx[wi, capA:].reshape(-1)
            db[:nB] = d_loc[segB]
            wa = wrelf[wi, :capA].reshape(-1)
            wa[:nA] = wrel_of[d_loc[segA]]
            wb = wrelf[wi, capA:].reshape(-1)
            wb[:nB] = wrel_of[d_loc[segB]]
        cores.append((srcidx.astype(np.uint16), dstidx.astype(np.uint16),
                      wrelf, pos))
    return cores, capA, capB


def _wrap_calls(idx_blocks, calls):
    """idx_blocks [NW, NB, 128] -> wrapped int16 [128, NW*NB*8] call-by-call."""
    NWl, NB, _ = idx_blocks.shape
    cols = []
    for wi in range(NWl):
        for (b0, b1) in calls:
            flat = idx_blocks[wi, b0:b1].reshape(-1)          # n = (b1-b0)*128
            wrapped = flat.reshape(-1, 16).T                  # [16, n/16]
            cols.append(np.tile(wrapped, (8, 1)))             # [128, n/16]
    return np.ascontiguousarray(np.concatenate(cols, axis=1).view(np.int16))


def _chunks(b0, b1):
    out = []
    b = b0
    while b < b1:
        out.append((b, min(b + CALL_BLOCKS, b1)))
        b = out[-1][1]
    return out


def _build_bass(capA, capB):
    import concourse.bass as bass
    import concourse.bacc as bacc
    import concourse.tile as tile
    import concourse.mybir as mybir
    from concourse.masks import make_identity

    f32 = mybir.dt.float32
    i16 = mybir.dt.int16
    i32 = mybir.dt.int32
    NB = capA + capB
    A_calls = _chunks(0, capA)
    B_calls = _chunks(capA, NB)
    D_calls = _chunks(0, NB)
    idx_cols = NW * NB * 8

    nc = bacc.Bacc("TRN2", target_bir_lowering=False, debug=False,
                   num_devices=NCORES, num_swdge_queues=4)

    xT = nc.dram_tensor("xT", [IN, HTAB_ROWS], f32, kind="ExternalInput")
    W_in = nc.dram_tensor("W", [IN, HC], f32, kind="ExternalInput")
    attblk = nc.dram_tensor("attblk", [HC, 2 * H], f32, kind="ExternalInput")
    bias_in = nc.dram_tensor("bias", [1, HC], f32, kind="ExternalInput")
    srcidx_in = nc.dram_tensor("srcidx", [128, idx_cols], i16, kind="ExternalInput")
    dstidx_in = nc.dram_tensor("dstidx", [128, idx_cols], i16, kind="ExternalInput")
    wrel_in = nc.dram_tensor("wrel", [128, NW * NB], f32, kind="ExternalInput")
    out_t = nc.dram_tensor("out", [NLOC, HC], f32, kind="ExternalOutput")
    htab = nc.dram_tensor("htab", [HTAB_ROWS, 128], f32)

    with tile.TileContext(nc) as tc:
        with tc.tile_pool(name="persist", bufs=1) as pp:
            w_ext = pp.tile([128, EXT], f32)
            sloc = pp.tile([128, NW, 68], f32)
            srcidx_sb = pp.tile([128, idx_cols], i16)
            dstidx_sb = pp.tile([128, idx_cols], i16)
            wrel_sb = pp.tile([128, NW * NB], f32)
            iota_f = pp.tile([128, 128], f32)
            bias_bc = pp.tile([128, HC], f32)
            ident = pp.tile([128, 128], f32)

            nc.sync.dma_start(srcidx_sb[:], srcidx_in[:])
            nc.sync.dma_start(dstidx_sb[:], dstidx_in[:])
            nc.sync.dma_start(wrel_sb[:], wrel_in[:])

            iota_i = pp.tile([128, 128], i32)
            nc.gpsimd.iota(iota_i[:], pattern=[[1, 128]], base=0,
                           channel_multiplier=0)
            nc.vector.tensor_copy(iota_f[:], iota_i[:])

            bias_row = pp.tile([1, HC], f32)
            nc.sync.dma_start(bias_row[:], bias_in[:])
            nc.gpsimd.partition_broadcast(bias_bc[:], bias_row[:])

            # ---- W_ext = [W | W@att_src_blk | W@att_dst_blk] -------------
            make_identity(nc, ident[:])
            nc.sync.dma_start(w_ext[:, 0:HC], W_in[:])
            attblk_sb = pp.tile([HC, 2 * H], f32)
            nc.sync.dma_start(attblk_sb[:], attblk[:])
            with (tc.tile_pool(name="wprep", bufs=1) as wp,
                  tc.tile_pool(name="wprep_ps", bufs=1, space="PSUM") as wpp):
                wt_ps = wpp.tile([HC, 128], f32, space="PSUM")
                nc.tensor.transpose(wt_ps[:], w_ext[:, 0:HC], ident[:])
                wt_sb = wp.tile([HC, 128], f32)
                nc.vector.tensor_copy(wt_sb[:], wt_ps[:])
                a_ps = wpp.tile([128, 2 * H], f32, space="PSUM")
                nc.tensor.matmul(a_ps[:], lhsT=wt_sb[:], rhs=attblk_sb[:],
                                 start=True, stop=True)
                nc.vector.tensor_copy(w_ext[:, HC:EXT], a_ps[:])

            # ---- Phase A: htab ------------------------------------------
            with (tc.tile_pool(name="pa", bufs=3) as pa,
                  tc.tile_pool(name="pa_ps", bufs=2, space="PSUM") as pap):
                groups = [(g * 768, 6) for g in range(65)] + [(49920, 3)]
                for gi, (r0, tpg) in enumerate(groups):
                    xt = pa.tile([128, 6 * 128], f32, tag="xt")
                    nc.sync.dma_start(
                        xt[:, :tpg * 128], xT[:, r0:r0 + tpg * 128])
                    hps = pap.tile([128, 6 * EXT], f32, space="PSUM",
                                   tag="hps")
                    for i in range(tpg):
                        nc.tensor.matmul(
                            hps[:, i * EXT:(i + 1) * EXT],
                            lhsT=xt[:, i * 128:(i + 1) * 128],
                            rhs=w_ext[:], start=True, stop=True)
                    hsb = pa.tile([128, 6, EXT], f32, tag="hsb")
                    if gi % 2 == 0:
                        nc.vector.tensor_copy(
                            hsb[:, :tpg].rearrange("p t c -> p (t c)"),
                            hps[:, :tpg * EXT])
                    else:
                        nc.scalar.copy(
                            hsb[:, :tpg].rearrange("p t c -> p (t c)"),
                            hps[:, :tpg * EXT])
                    nc.sync.dma_start(
                        htab[r0:r0 + tpg * 128, 0:EXT]
                        .rearrange("(t p) c -> p t c", p=128),
                        hsb[:, :tpg])

            # ---- Phase B: edge aggregation ------------------------------
            htabA = htab[0:SPLIT, 0:128]
            htabB = htab[SPLIT:HTAB_ROWS, 0:128]
            htabD = htab[0:NLOC, 64:128]
            with (tc.tile_pool(name="pb", bufs=3) as pb,
                  tc.tile_pool(name="pb_oh", bufs=20) as pboh,
                  tc.tile_pool(name="pb_ps", bufs=4, space="PSUM") as pbp):
                for w in range(NW):
                    gs = pb.tile([128, NB, 128], f32, tag="gs")
                    gd = pb.tile([128, NB, 64], f32, tag="gd")
                    col0 = w * NB * 8
                    qn = 0
                    for (b0, b1) in A_calls:
                        n = (b1 - b0) * 128
                        nc.gpsimd.dma_gather(
                            gs[:, b0:b1], htabA,
                            srcidx_sb[:, col0 + b0 * 8: col0 + b1 * 8],
                            num_idxs=n, num_idxs_reg=n, elem_size=128,
                            queue_num=qn % 4)
                        qn += 1
                    for (b0, b1) in B_calls:
                        n = (b1 - b0) * 128
                        nc.gpsimd.dma_gather(
                            gs[:, b0:b1], htabB,
                            srcidx_sb[:, col0 + b0 * 8: col0 + b1 * 8],
                            num_idxs=n, num_idxs_reg=n, elem_size=128,
                            queue_num=qn % 4)
                        qn += 1
                    for (b0, b1) in D_calls:
                        n = (b1 - b0) * 128
                        nc.gpsimd.dma_gather(
                            gd[:, b0:b1], htabD,
                            dstidx_sb[:, col0 + b0 * 8: col0 + b1 * 8],
                            num_idxs=n, num_idxs_reg=n, elem_size=64,
                            elem_step=128, queue_num=qn % 4)
                        qn += 1
                    et = pb.tile([128, NB, H], f32, tag="et")
                    nc.vector.tensor_tensor(
                        out=et[:], in0=gs[:, :, 64:68], in1=gd[:, :, 4:8],
                        op=bass.mybir.AluOpType.add)
                    # leaky_relu(z) = max(z, 0.2*z), exact on DVE
                    et2 = pb.tile([128, NB, H], f32, tag="et2")
                    nc.vector.tensor_scalar_mul(et2[:], et[:], NEG_SLOPE)
                    nc.vector.tensor_tensor(
                        out=et[:], in0=et[:], in1=et2[:],
                        op=bass.mybir.AluOpType.max)
                    nc.scalar.activation(
                        gs[:, :, 64:68], et[:],
                        bass.mybir.ActivationFunctionType.Exp)
                    # V = p (x) h, in place on gs cols 0:64
                    pexp = gs[:, :, 64:68]
                    p_bc = bass.AP(pexp.tensor, pexp.offset,
                                   [pexp.ap[0], pexp.ap[1], pexp.ap[2],
                                    [0, C]])
                    nc.vector.tensor_tensor(
                        out=gs[:, :, 0:64].rearrange("p b (h c) -> p b h c",
                                                     c=C),
                        in0=gs[:, :, 0:64].rearrange("p b (h c) -> p b h c",
                                                     c=C),
                        in1=p_bc, op=bass.mybir.AluOpType.mult)
                    ps = pbp.tile([128, 68], f32, space="PSUM", tag="pw")
                    for b in range(NB):
                        oh = pboh.tile([128, 128], f32, tag="oh")
                        nc.vector.tensor_scalar(
                            out=oh[:], in0=iota_f[:],
                            scalar1=wrel_sb[:, w * NB + b: w * NB + b + 1],
                            scalar2=None,
                            op0=bass.mybir.AluOpType.is_equal)
                        nc.tensor.matmul(ps[:], lhsT=oh[:],
                                         rhs=gs[:, b, 0:68],
                                         start=(b == 0), stop=(b == NB - 1))
                    nc.scalar.copy(sloc[:, w, :], ps[:])

            # ---- Phase C: normalize + bias ------------------------------
            with tc.tile_pool(name="pc", bufs=2) as pc:
                for w in range(NW):
                    dt = pc.tile([128, H], f32, tag="dt")
                    nc.vector.tensor_scalar_add(dt[:], sloc[:, w, 64:68],
                                                1e-16)
                    rc = pc.tile([128, H], f32, tag="rc")
                    nc.vector.reciprocal(rc[:], dt[:])
                    rbc = bass.AP(rc[:].tensor, rc[:].offset,
                                  [rc[:].ap[0], rc[:].ap[1], [0, C]])
                    ot = pc.tile([128, HC], f32, tag="ot")
                    nc.vector.tensor_tensor(
                        out=ot[:].rearrange("p (h c) -> p h c", c=C),
                        in0=sloc[:, w, 0:64].rearrange("p (h c) -> p h c",
                                                       c=C),
                        in1=rbc, op=bass.mybir.AluOpType.mult)
                    nc.vector.tensor_tensor(out=ot[:], in0=ot[:],
                                            in1=bias_bc[:],
                                            op=bass.mybir.AluOpType.add)
                    nc.sync.dma_start(out_t[w * 128:(w + 1) * 128, :], ot[:])

    nc.compile()
    return nc


def kernel(x, edge_index, W, att_src, att_dst, bias):
    from concourse.bass_utils import run_bass_kernel_spmd

    x = np.asarray(x)
    edge_index = np.asarray(edge_index)
    W = np.asarray(W)
    att_src = np.asarray(att_src)
    att_dst = np.asarray(att_dst)
    bias = np.asarray(bias)

    cores, capA, capB = _build_host_data(edge_index)
    NB = capA + capB

    key = (capA, capB)
    if key not in _cache:
        _cache[key] = _build_bass(capA, capB)
    nc = _cache[key]

    # host-side layout prep (no model math: att_blk places att values into a
    # block-diagonal zero matrix; a_src/a_dst projections are computed on
    # device as W @ att_blk)
    attblk = np.zeros((HC, 2 * H), np.float32)
    for h in range(H):
        attblk[h * C:(h + 1) * C, h] = att_src[h]
        attblk[h * C:(h + 1) * C, H + h] = att_dst[h]

    xT_full = np.zeros((IN, HTAB_ROWS), np.float32)
    xT_full[:, :N] = x.T

    A_calls = _chunks(0, capA)
    B_calls = _chunks(capA, NB)
    D_calls = _chunks(0, NB)

    in_maps = []
    perms = []
    for c in range(NCORES):
        srcidx, dstidx, wrelf, pos = cores[c]
        perms.append(pos)
        xT_c = np.roll(xT_full[:, :NPAD], -c * NLOC, axis=1)
        xT_c = np.concatenate(
            [xT_c, np.zeros((IN, HTAB_ROWS - NPAD), np.float32)], axis=1)
        in_maps.append({
            "xT": np.ascontiguousarray(xT_c),
            "W": W.astype(np.float32),
            "attblk": attblk,
            "bias": bias.astype(np.float32).reshape(1, HC),
            "srcidx": _wrap_calls(srcidx, A_calls + B_calls),
            "dstidx": _wrap_calls(dstidx, D_calls),
            "wrel": np.ascontiguousarray(
                wrelf.transpose(2, 0, 1).reshape(128, NW * NB)),
        })

    res = run_bass_kernel_spmd(nc, in_maps, core_ids=list(range(NCORES)))
    global LAST_RESULTS
    LAST_RESULTS = res
    out = np.concatenate(
        [res.results[c]["out"][perms[c]] for c in range(NCORES)], axis=0)
    return np.ascontiguousarray(out[:N])


LAST_RESULTS = None


# revision 12
# speedup vs baseline: 1.0467x; 1.0467x over previous
"""GAT (graph attention) convolution on 8 Trainium2 NeuronCores.

Strategy (all-standard + validated primitives):
  - Destination-shard the edges: core c owns dst nodes [c*6272, (c+1)*6272).
    Each core's x input is ROTATED so its own nodes are table rows 0..6271
    (keeps all dma_gather int16 indices in range; gather tables are split
    at row 32768 for the signed-int16 limit).
  - Phase A (per core): h_ext = x @ [W | W@att_src_blk | W@att_dst_blk]
    -> DRAM table htab[rows, 128] = [h(64) | a_src(4) | a_dst(4) | junk].
  - Phase B: edges sorted by dst window (128 dst nodes per window).
    dma_gather h_ext rows by src (512B rows) and a-values by dst (256B).
    p = exp(leaky_relu(a_src[src] + a_dst[dst])); V = [p*h | p].
    Segment-sum into the window via one-hot matmuls accumulated in PSUM
    (one-hot built with tensor_scalar is_equal against an iota row).
  - Phase C: out = S/(D + 1e-16) + bias for the core's own 6272 nodes.

No max-subtraction in the softmax: e has tiny dynamic range (|e| < ~8), so
exp(e)/sum(exp(e)) matches the reference's exp(e-m)/sum(exp(e-m)) to fp32
rounding.
"""
import numpy as np

# ---------------------------------------------------------------- constants
N = 50000
E = 800000
IN = 128
H = 4
C = 16
HC = H * C            # 64
EXT = 72              # h(64) + a_src(4) + a_dst(4)
NEG_SLOPE = 0.2
NCORES = 8
NLOC = 6272           # 49*128 nodes per core
NW = NLOC // 128      # 49 windows per core
NPAD = NLOC * NCORES  # 50176
HTAB_ROWS = 50304     # 393*128 = NPAD + 128
SPLIT = 32768         # int16 gather index limit
CALL_BLOCKS = 8       # 8*128 = 1024 idxs per dma_gather call (SWDGE ring cap)

_cache = {}


def _build_host_data(edge_index):
    """Sort/shard/pad edges; returns per-core index/wrel arrays and caps."""
    src = edge_index[0].astype(np.int64)
    dst = edge_index[1].astype(np.int64)
    # self-loops as ordinary edges
    loops = np.arange(N, dtype=np.int64)
    src = np.concatenate([src, loops])
    dst = np.concatenate([dst, loops])

    core = dst // NLOC
    per_core = []
    perms = []
    capA = capB = 0
    for c in range(NCORES):
        m = core == c
        s_rot = (src[m] - c * NLOC) % NPAD
        d_loc = dst[m] - c * NLOC
        # balance windows: assign nodes to windows greedily by degree so every
        # window gets ~equal edge count (output rows are permuted; undone on
        # the host after the run)
        isB = s_rot >= SPLIT
        degA = np.bincount(d_loc[~isB], minlength=NLOC)
        degB = np.bincount(d_loc[isB], minlength=NLOC)
        deg = degA + degB
        order_n = np.argsort(-deg, kind="stable")
        wcntA = np.zeros(NW, np.int64)
        wcntB = np.zeros(NW, np.int64)
        wfill = np.zeros(NW, np.int64)
        pos = np.zeros(NLOC, np.int64)     # node -> device row (w*128+wrel)
        for nd in order_n:
            # balance the worse of the two gather groups across windows
            score = np.maximum((wcntA + degA[nd]) / 1536.0,
                               (wcntB + degB[nd]) / 768.0)
            score[wfill >= 128] = 1e9
            wi = int(np.argmin(score))
            pos[nd] = wi * 128 + wfill[wi]
            wcntA[wi] += degA[nd]
            wcntB[wi] += degB[nd]
            wfill[wi] += 1
        perms.append(pos)
        w = pos[d_loc] >> 7
        grp = (s_rot >= SPLIT).astype(np.int64)
        order = np.lexsort((s_rot, grp, w))
        s_rot, d_loc, w, grp = s_rot[order], d_loc[order], w[order], grp[order]
        cntA = np.bincount(w[grp == 0], minlength=NW)
        cntB = np.bincount(w[grp == 1], minlength=NW)
        capA = max(capA, int(np.ceil(cntA.max() / 128)))
        capB = max(capB, int(np.ceil(cntB.max() / 128)))
        per_core.append((s_rot, d_loc, w, grp, cntA, cntB, pos))

    NB = capA + capB
    cores = []
    for c in range(NCORES):
        s_rot, d_loc, w, grp, cntA, cntB, pos = per_core[c]
        wrel_of = pos & 127
        srcidx = np.zeros((NW, NB, 128), np.int64)
        dstidx = np.zeros((NW, NB, 128), np.int64)
        wrelf = np.full((NW, NB, 128), 999.0, np.float32)
        # edges are sorted by (w, grp); compute start offset of each (w,grp)
        startA = np.concatenate([[0], np.cumsum(cntA + cntB)[:-1]])
        for wi in range(NW):
            a0 = startA[wi]
            nA = cntA[wi]
            nB = cntB[wi]
            segA = slice(a0, a0 + nA)
            segB = slice(a0 + nA, a0 + nA + nB)
            fa = srcidx[wi, :capA].reshape(-1)
            fa[:nA] = s_rot[segA]
            fb = srcidx[wi, capA:].reshape(-1)
            fb[:nB] = s_rot[segB] - SPLIT
            da = dstidx[wi, :capA].reshape(-1)
            da[:nA] = d_loc[segA]
            db = dstidx[wi, capA:].reshape(-1)
            db[:nB] = d_loc[segB]
            wa = wrelf[wi, :capA].reshape(-1)
            wa[:nA] = wrel_of[d_loc[segA]]
            wb = wrelf[wi, capA:].reshape(-1)
            wb[:nB] = wrel_of[d_loc[segB]]
        cores.append((srcidx.astype(np.uint16), dstidx.astype(np.uint16),
                      wrelf, pos))
    return cores, capA, capB


def _wrap_calls(idx_blocks, calls):
    """idx_blocks [NW, NB, 128] -> wrapped int16 [128, NW*NB*8] call-by-call."""
    NWl, NB, _ = idx_blocks.shape
    cols = []
    for wi in range(NWl):
        for (b0, b1) in calls:
            flat = idx_blocks[wi, b0:b1].reshape(-1)          # n = (b1-b0)*128
            wrapped = flat.reshape(-1, 16).T                  # [16, n/16]
            cols.append(np.tile(wrapped, (8, 1)))             # [128, n/16]
    return np.ascontiguousarray(np.concatenate(cols, axis=1).view(np.int16))


def _chunks(b0, b1):
    out = []
    b = b0
    while b < b1:
        out.append((b, min(b + CALL_BLOCKS, b1)))
        b = out[-1][1]
    return out


def _build_bass(capA, capB):
    import concourse.bass as bass
    import concourse.bacc as bacc
    import concourse.tile as tile
    import concourse.mybir as mybir
    from concourse.masks import make_identity

    f32 = mybir.dt.float32
    i16 = mybir.dt.int16
    i32 = mybir.dt.int32
    NB = capA + capB
    A_calls = _chunks(0, capA)
    B_calls = _chunks(capA, NB)
    D_calls = _chunks(0, NB)
    idx_cols = NW * NB * 8

    nc = bacc.Bacc("TRN2", target_bir_lowering=False, debug=False,
                   num_devices=NCORES, num_swdge_queues=4)

    xT = nc.dram_tensor("xT", [IN, HTAB_ROWS], f32, kind="ExternalInput")
    W_in = nc.dram_tensor("W", [IN, HC], f32, kind="ExternalInput")
    attblk = nc.dram_tensor("attblk", [HC, 2 * H], f32, kind="ExternalInput")
    bias_in = nc.dram_tensor("bias", [1, HC], f32, kind="ExternalInput")
    srcidx_in = nc.dram_tensor("srcidx", [128, idx_cols], i16, kind="ExternalInput")
    dstidx_in = nc.dram_tensor("dstidx", [128, idx_cols], i16, kind="ExternalInput")
    wrel_in = nc.dram_tensor("wrel", [128, NW * NB], f32, kind="ExternalInput")
    out_t = nc.dram_tensor("out", [NLOC, HC], f32, kind="ExternalOutput")
    htab = nc.dram_tensor("htab", [HTAB_ROWS, 128], f32)

    with tile.TileContext(nc) as tc:
        with tc.tile_pool(name="persist", bufs=1) as pp:
            w_ext = pp.tile([128, EXT], f32)
            sloc = pp.tile([128, NW, 68], f32)
            srcidx_sb = pp.tile([128, idx_cols], i16)
            dstidx_sb = pp.tile([128, idx_cols], i16)
            wrel_sb = pp.tile([128, NW * NB], f32)
            iota_f = pp.tile([128, 128], f32)
            bias_bc = pp.tile([128, HC], f32)
            ident = pp.tile([128, 128], f32)

            nc.sync.dma_start(srcidx_sb[:], srcidx_in[:])
            nc.sync.dma_start(dstidx_sb[:], dstidx_in[:])
            nc.sync.dma_start(wrel_sb[:], wrel_in[:])

            iota_i = pp.tile([128, 128], i32)
            nc.gpsimd.iota(iota_i[:], pattern=[[1, 128]], base=0,
                           channel_multiplier=0)
            nc.vector.tensor_copy(iota_f[:], iota_i[:])

            bias_row = pp.tile([1, HC], f32)
            nc.sync.dma_start(bias_row[:], bias_in[:])
            nc.gpsimd.partition_broadcast(bias_bc[:], bias_row[:])

            # ---- W_ext = [W | W@att_src_blk | W@att_dst_blk] -------------
            make_identity(nc, ident[:])
            nc.sync.dma_start(w_ext[:, 0:HC], W_in[:])
            attblk_sb = pp.tile([HC, 2 * H], f32)
            nc.sync.dma_start(attblk_sb[:], attblk[:])
            with (tc.tile_pool(name="wprep", bufs=1) as wp,
                  tc.tile_pool(name="wprep_ps", bufs=1, space="PSUM") as wpp):
                wt_ps = wpp.tile([HC, 128], f32, space="PSUM")
                nc.tensor.transpose(wt_ps[:], w_ext[:, 0:HC], ident[:])
                wt_sb = wp.tile([HC, 128], f32)
                nc.vector.tensor_copy(wt_sb[:], wt_ps[:])
                a_ps = wpp.tile([128, 2 * H], f32, space="PSUM")
                nc.tensor.matmul(a_ps[:], lhsT=wt_sb[:], rhs=attblk_sb[:],
                                 start=True, stop=True)
                nc.vector.tensor_copy(w_ext[:, HC:EXT], a_ps[:])

            # ---- Phase A: htab ------------------------------------------
            with (tc.tile_pool(name="pa", bufs=3) as pa,
                  tc.tile_pool(name="pa_ps", bufs=2, space="PSUM") as pap):
                groups = [(g * 768, 6) for g in range(65)] + [(49920, 3)]
                for gi, (r0, tpg) in enumerate(groups):
                    xt = pa.tile([128, 6 * 128], f32, tag="xt")
                    nc.sync.dma_start(
                        xt[:, :tpg * 128], xT[:, r0:r0 + tpg * 128])
                    hps = pap.tile([128, 6 * EXT], f32, space="PSUM",
                                   tag="hps")
                    for i in range(tpg):
                        nc.tensor.matmul(
                            hps[:, i * EXT:(i + 1) * EXT],
                            lhsT=xt[:, i * 128:(i + 1) * 128],
                            rhs=w_ext[:], start=True, stop=True)
                    hsb = pa.tile([128, 6, EXT], f32, tag="hsb")
                    if gi % 2 == 0:
                        nc.vector.tensor_copy(
                            hsb[:, :tpg].rearrange("p t c -> p (t c)"),
                            hps[:, :tpg * EXT])
                    else:
                        nc.scalar.copy(
                            hsb[:, :tpg].rearrange("p t c -> p (t c)"),
                            hps[:, :tpg * EXT])
                    nc.sync.dma_start(
                        htab[r0:r0 + tpg * 128, 0:EXT]
                        .rearrange("(t p) c -> p t c", p=128),
                        hsb[:, :tpg])

            # ---- Phase B: edge aggregation ------------------------------
            htabA = htab[0:SPLIT, 0:128]
            htabB = htab[SPLIT:HTAB_ROWS, 0:128]
            htabD = htab[0:NLOC, 64:128]
            with (tc.tile_pool(name="pb", bufs=3) as pb,
                  tc.tile_pool(name="pb_oh", bufs=20) as pboh,
                  tc.tile_pool(name="pb_ps", bufs=4, space="PSUM") as pbp):
                for w in range(NW):
                    gs = pb.tile([128, NB, 128], f32, tag="gs")
                    gd = pb.tile([128, NB, 64], f32, tag="gd")
                    col0 = w * NB * 8
                    qn = 0
                    for (b0, b1) in A_calls:
                        n = (b1 - b0) * 128
                        nc.gpsimd.dma_gather(
                            gs[:, b0:b1], htabA,
                            srcidx_sb[:, col0 + b0 * 8: col0 + b1 * 8],
                            num_idxs=n, num_idxs_reg=n, elem_size=128,
                            queue_num=qn % 4)
                        qn += 1
                    for (b0, b1) in B_calls:
                        n = (b1 - b0) * 128
                        nc.gpsimd.dma_gather(
                            gs[:, b0:b1], htabB,
                            srcidx_sb[:, col0 + b0 * 8: col0 + b1 * 8],
                            num_idxs=n, num_idxs_reg=n, elem_size=128,
                            queue_num=qn % 4)
                        qn += 1
                    for (b0, b1) in D_calls:
                        n = (b1 - b0) * 128
                        nc.gpsimd.dma_gather(
                            gd[:, b0:b1], htabD,
                            dstidx_sb[:, col0 + b0 * 8: col0 + b1 * 8],
                            num_idxs=n, num_idxs_reg=n, elem_size=64,
                            elem_step=128, queue_num=qn % 4)
                        qn += 1
                    et = pb.tile([128, NB, H], f32, tag="et")
                    nc.vector.tensor_tensor(
                        out=et[:], in0=gs[:, :, 64:68], in1=gd[:, :, 4:8],
                        op=bass.mybir.AluOpType.add)
                    # leaky_relu(z) = max(z, 0.2*z), exact on DVE
                    et2 = pb.tile([128, NB, H], f32, tag="et2")
                    nc.vector.tensor_scalar_mul(et2[:], et[:], NEG_SLOPE)
                    nc.vector.tensor_tensor(
                        out=et[:], in0=et[:], in1=et2[:],
                        op=bass.mybir.AluOpType.max)
                    nc.scalar.activation(
                        gs[:, :, 64:68], et[:],
                        bass.mybir.ActivationFunctionType.Exp)
                    # V = p (x) h, in place on gs cols 0:64
                    pexp = gs[:, :, 64:68]
                    p_bc = bass.AP(pexp.tensor, pexp.offset,
                                   [pexp.ap[0], pexp.ap[1], pexp.ap[2],
                                    [0, C]])
                    nc.vector.tensor_tensor(
                        out=gs[:, :, 0:64].rearrange("p b (h c) -> p b h c",
                                                     c=C),
                        in0=gs[:, :, 0:64].rearrange("p b (h c) -> p b h c",
                                                     c=C),
                        in1=p_bc, op=bass.mybir.AluOpType.mult)
                    ps = pbp.tile([128, 68], f32, space="PSUM", tag="pw")
                    for b in range(NB):
                        oh = pboh.tile([128, 128], f32, tag="oh")
                        nc.vector.tensor_scalar(
                            out=oh[:], in0=iota_f[:],
                            scalar1=wrel_sb[:, w * NB + b: w * NB + b + 1],
                            scalar2=None,
                            op0=bass.mybir.AluOpType.is_equal)
                        nc.tensor.matmul(ps[:], lhsT=oh[:],
                                         rhs=gs[:, b, 0:68],
                                         start=(b == 0), stop=(b == NB - 1))
                    nc.scalar.copy(sloc[:, w, :], ps[:])

            # ---- Phase C: normalize + bias ------------------------------
            with tc.tile_pool(name="pc", bufs=2) as pc:
                for w in range(NW):
                    dt = pc.tile([128, H], f32, tag="dt")
                    nc.vector.tensor_scalar_add(dt[:], sloc[:, w, 64:68],
                                                1e-16)
                    rc = pc.tile([128, H], f32, tag="rc")
                    nc.vector.reciprocal(rc[:], dt[:])
                    rbc = bass.AP(rc[:].tensor, rc[:].offset,
                                  [rc[:].ap[0], rc[:].ap[1], [0, C]])
                    ot = pc.tile([128, HC], f32, tag="ot")
                    nc.vector.tensor_tensor(
                        out=ot[:].rearrange("p (h c) -> p h c", c=C),
                        in0=sloc[:, w, 0:64].rearrange("p (h c) -> p h c",
                                                       c=C),
                        in1=rbc, op=bass.mybir.AluOpType.mult)
                    nc.vector.tensor_tensor(out=ot[:], in0=ot[:],
                                            in1=bias_bc[:],
                                            op=bass.mybir.AluOpType.add)
                    nc.sync.dma_start(out_t[w * 128:(w + 1) * 128, :], ot[:])

    nc.compile()
    return nc


def kernel(x, edge_index, W, att_src, att_dst, bias):
    from concourse.bass_utils import run_bass_kernel_spmd

    x = np.asarray(x)
    edge_index = np.asarray(edge_index)
    W = np.asarray(W)
    att_src = np.asarray(att_src)
    att_dst = np.asarray(att_dst)
    bias = np.asarray(bias)

    cores, capA, capB = _build_host_data(edge_index)
    NB = capA + capB

    key = (capA, capB)
    if key not in _cache:
        _cache[key] = _build_bass(capA, capB)
    nc = _cache[key]

    # host-side layout prep (no model math: att_blk places att values into a
    # block-diagonal zero matrix; a_src/a_dst projections are computed on
    # device as W @ att_blk)
    attblk = np.zeros((HC, 2 * H), np.float32)
    for h in range(H):
        attblk[h * C:(h + 1) * C, h] = att_src[h]
        attblk[h * C:(h + 1) * C, H + h] = att_dst[h]

    xT_full = np.zeros((IN, HTAB_ROWS), np.float32)
    xT_full[:, :N] = x.T

    A_calls = _chunks(0, capA)
    B_calls = _chunks(capA, NB)
    D_calls = _chunks(0, NB)

    in_maps = []
    perms = []
    for c in range(NCORES):
        srcidx, dstidx, wrelf, pos = cores[c]
        perms.append(pos)
        xT_c = np.roll(xT_full[:, :NPAD], -c * NLOC, axis=1)
        xT_c = np.concatenate(
            [xT_c, np.zeros((IN, HTAB_ROWS - NPAD), np.float32)], axis=1)
        in_maps.append({
            "xT": np.ascontiguousarray(xT_c),
            "W": W.astype(np.float32),
            "attblk": attblk,
            "bias": bias.astype(np.float32).reshape(1, HC),
            "srcidx": _wrap_calls(srcidx, A_calls + B_calls),
            "dstidx": _wrap_calls(dstidx, D_calls),
            "wrel": np.ascontiguousarray(
                wrelf.transpose(2, 0, 1).reshape(128, NW * NB)),
        })

    res = run_bass_kernel_spmd(nc, in_maps, core_ids=list(range(NCORES)))
    global LAST_RESULTS
    LAST_RESULTS = res
    out = np.concatenate(
        [res.results[c]["out"][perms[c]] for c in range(NCORES)], axis=0)
    return np.ascontiguousarray(out[:N])


LAST_RESULTS = None


# revision 13
# speedup vs baseline: 1.1081x; 1.0587x over previous
"""GAT (graph attention) convolution on 8 Trainium2 NeuronCores.

Strategy (all-standard + validated primitives):
  - Destination-shard the edges: core c owns dst nodes [c*6272, (c+1)*6272).
    Each core's x input is ROTATED so its own nodes are table rows 0..6271
    (keeps all dma_gather int16 indices in range; gather tables are split
    at row 32768 for the signed-int16 limit).
  - Phase A (per core): h_ext = x @ [W | W@att_src_blk | W@att_dst_blk]
    -> DRAM table htab[rows, 128] = [h(64) | a_src(4) | a_dst(4) | junk].
  - Phase B: edges sorted by dst window (128 dst nodes per window).
    dma_gather h_ext rows by src (512B rows) and a-values by dst (256B).
    p = exp(leaky_relu(a_src[src] + a_dst[dst])); V = [p*h | p].
    Segment-sum into the window via one-hot matmuls accumulated in PSUM
    (one-hot built with tensor_scalar is_equal against an iota row).
  - Phase C: out = S/(D + 1e-16) + bias for the core's own 6272 nodes.

No max-subtraction in the softmax: e has tiny dynamic range (|e| < ~8), so
exp(e)/sum(exp(e)) matches the reference's exp(e-m)/sum(exp(e-m)) to fp32
rounding.
"""
import numpy as np

# ---------------------------------------------------------------- constants
N = 50000
E = 800000
IN = 128
H = 4
C = 16
HC = H * C            # 64
EXT = 72              # h(64) + a_src(4) + a_dst(4)
NEG_SLOPE = 0.2
NCORES = 8
NLOC = 6272           # 49*128 nodes per core
NW = NLOC // 128      # 49 windows per core
NPAD = NLOC * NCORES  # 50176
HTAB_ROWS = 50304     # 393*128 = NPAD + 128
SPLIT = 32768         # int16 gather index limit
CALL_BLOCKS = 8       # 8*128 = 1024 idxs per dma_gather call (SWDGE ring cap)

_cache = {}


def _build_host_data(edge_index):
    """Sort/shard/pad edges; returns per-core index/wrel arrays and caps."""
    src = edge_index[0].astype(np.int64)
    dst = edge_index[1].astype(np.int64)
    # self-loops as ordinary edges
    loops = np.arange(N, dtype=np.int64)
    src = np.concatenate([src, loops])
    dst = np.concatenate([dst, loops])

    core = dst // NLOC
    per_core = []
    perms = []
    capA = capB = 0
    for c in range(NCORES):
        m = core == c
        s_rot = (src[m] - c * NLOC) % NPAD
        d_loc = dst[m] - c * NLOC
        # balance windows: assign nodes to windows greedily by degree so every
        # window gets ~equal edge count (output rows are permuted; undone on
        # the host after the run)
        isB = s_rot >= SPLIT
        degA = np.bincount(d_loc[~isB], minlength=NLOC)
        degB = np.bincount(d_loc[isB], minlength=NLOC)
        deg = degA + degB
        order_n = np.argsort(-deg, kind="stable")
        wcntA = np.zeros(NW, np.int64)
        wcntB = np.zeros(NW, np.int64)
        wfill = np.zeros(NW, np.int64)
        pos = np.zeros(NLOC, np.int64)     # node -> device row (w*128+wrel)
        for nd in order_n:
            # balance the worse of the two gather groups across windows
            score = np.maximum((wcntA + degA[nd]) / 1536.0,
                               (wcntB + degB[nd]) / 768.0)
            score[wfill >= 128] = 1e9
            wi = int(np.argmin(score))
            pos[nd] = wi * 128 + wfill[wi]
            wcntA[wi] += degA[nd]
            wcntB[wi] += degB[nd]
            wfill[wi] += 1
        perms.append(pos)
        w = pos[d_loc] >> 7
        grp = (s_rot >= SPLIT).astype(np.int64)
        order = np.lexsort((s_rot, grp, w))
        s_rot, d_loc, w, grp = s_rot[order], d_loc[order], w[order], grp[order]
        cntA = np.bincount(w[grp == 0], minlength=NW)
        cntB = np.bincount(w[grp == 1], minlength=NW)
        capA = max(capA, int(np.ceil(cntA.max() / 128)))
        capB = max(capB, int(np.ceil(cntB.max() / 128)))
        per_core.append((s_rot, d_loc, w, grp, cntA, cntB, pos))

    NB = capA + capB
    cores = []
    for c in range(NCORES):
        s_rot, d_loc, w, grp, cntA, cntB, pos = per_core[c]
        wrel_of = pos & 127
        srcidx = np.zeros((NW, NB, 128), np.int64)
        dstidx = np.zeros((NW, NB, 128), np.int64)
        wrelf = np.full((NW, NB, 128), 999.0, np.float32)
        # edges are sorted by (w, grp); compute start offset of each (w,grp)
        startA = np.concatenate([[0], np.cumsum(cntA + cntB)[:-1]])
        for wi in range(NW):
            a0 = startA[wi]
            nA = cntA[wi]
            nB = cntB[wi]
            segA = slice(a0, a0 + nA)
            segB = slice(a0 + nA, a0 + nA + nB)
            fa = srcidx[wi, :capA].reshape(-1)
            fa[:nA] = s_rot[segA]
            fb = srcidx[wi, capA:].reshape(-1)
            fb[:nB] = s_rot[segB] - SPLIT
            da = dstidx[wi, :capA].reshape(-1)
            da[:nA] = d_loc[segA]
            db = dstidx[wi, capA:].reshape(-1)
            db[:nB] = d_loc[segB]
            wa = wrelf[wi, :capA].reshape(-1)
            wa[:nA] = wrel_of[d_loc[segA]]
            wb = wrelf[wi, capA:].reshape(-1)
            wb[:nB] = wrel_of[d_loc[segB]]
        cores.append((srcidx.astype(np.uint16), dstidx.astype(np.uint16),
                      wrelf, pos))
    return cores, capA, capB


def _wrap_calls(idx_blocks, calls):
    """idx_blocks [NW, NB, 128] -> wrapped int16 [128, NW*NB*8] call-by-call."""
    NWl, NB, _ = idx_blocks.shape
    cols = []
    for wi in range(NWl):
        for (b0, b1) in calls:
            flat = idx_blocks[wi, b0:b1].reshape(-1)          # n = (b1-b0)*128
            wrapped = flat.reshape(-1, 16).T                  # [16, n/16]
            cols.append(np.tile(wrapped, (8, 1)))             # [128, n/16]
    return np.ascontiguousarray(np.concatenate(cols, axis=1).view(np.int16))


def _chunks(b0, b1):
    out = []
    b = b0
    while b < b1:
        out.append((b, min(b + CALL_BLOCKS, b1)))
        b = out[-1][1]
    return out


def _build_bass(capA, capB):
    import concourse.bass as bass
    import concourse.bacc as bacc
    import concourse.tile as tile
    import concourse.mybir as mybir
    from concourse.masks import make_identity

    f32 = mybir.dt.float32
    i16 = mybir.dt.int16
    i32 = mybir.dt.int32
    NB = capA + capB
    A_calls = _chunks(0, capA)
    B_calls = _chunks(capA, NB)
    D_calls = _chunks(0, NB)
    idx_cols = NW * NB * 8

    nc = bacc.Bacc("TRN2", target_bir_lowering=False, debug=False,
                   num_devices=NCORES, num_swdge_queues=4)

    xT = nc.dram_tensor("xT", [IN, HTAB_ROWS], f32, kind="ExternalInput")
    W_in = nc.dram_tensor("W", [IN, HC], f32, kind="ExternalInput")
    attblk = nc.dram_tensor("attblk", [HC, 2 * H], f32, kind="ExternalInput")
    bias_in = nc.dram_tensor("bias", [1, HC], f32, kind="ExternalInput")
    srcidx_in = nc.dram_tensor("srcidx", [128, idx_cols], i16, kind="ExternalInput")
    dstidx_in = nc.dram_tensor("dstidx", [128, idx_cols], i16, kind="ExternalInput")
    wrel_in = nc.dram_tensor("wrel", [128, NW * NB], f32, kind="ExternalInput")
    out_t = nc.dram_tensor("out", [NLOC, HC], f32, kind="ExternalOutput")
    htab = nc.dram_tensor("htab", [HTAB_ROWS, 128], f32)
    htab_own = nc.dram_tensor("htab_own", [6912, 128], f32)

    with tile.TileContext(nc) as tc:
        with tc.tile_pool(name="persist", bufs=1) as pp:
            w_ext = pp.tile([128, EXT], f32)
            sloc = pp.tile([128, NW, 68], f32)
            srcidx_sb = pp.tile([128, idx_cols], i16)
            dstidx_sb = pp.tile([128, idx_cols], i16)
            wrel_sb = pp.tile([128, NW * NB], f32)
            iota_f = pp.tile([128, 128], f32)
            bias_bc = pp.tile([128, HC], f32)
            ident = pp.tile([128, 128], f32)

            nc.sync.dma_start(srcidx_sb[:], srcidx_in[:])
            nc.sync.dma_start(dstidx_sb[:], dstidx_in[:])
            nc.sync.dma_start(wrel_sb[:], wrel_in[:])

            iota_i = pp.tile([128, 128], i32)
            nc.gpsimd.iota(iota_i[:], pattern=[[1, 128]], base=0,
                           channel_multiplier=0)
            nc.vector.tensor_copy(iota_f[:], iota_i[:])

            bias_row = pp.tile([1, HC], f32)
            nc.sync.dma_start(bias_row[:], bias_in[:])
            nc.gpsimd.partition_broadcast(bias_bc[:], bias_row[:])

            # ---- W_ext = [W | W@att_src_blk | W@att_dst_blk] -------------
            make_identity(nc, ident[:])
            nc.sync.dma_start(w_ext[:, 0:HC], W_in[:])
            attblk_sb = pp.tile([HC, 2 * H], f32)
            nc.sync.dma_start(attblk_sb[:], attblk[:])
            with (tc.tile_pool(name="wprep", bufs=1) as wp,
                  tc.tile_pool(name="wprep_ps", bufs=1, space="PSUM") as wpp):
                wt_ps = wpp.tile([HC, 128], f32, space="PSUM")
                nc.tensor.transpose(wt_ps[:], w_ext[:, 0:HC], ident[:])
                wt_sb = wp.tile([HC, 128], f32)
                nc.vector.tensor_copy(wt_sb[:], wt_ps[:])
                a_ps = wpp.tile([128, 2 * H], f32, space="PSUM")
                nc.tensor.matmul(a_ps[:], lhsT=wt_sb[:], rhs=attblk_sb[:],
                                 start=True, stop=True)
                nc.vector.tensor_copy(w_ext[:, HC:EXT], a_ps[:])

            # ---- Phase A: htab ------------------------------------------
            with (tc.tile_pool(name="pa", bufs=3) as pa,
                  tc.tile_pool(name="pa_ps", bufs=2, space="PSUM") as pap):
                groups = [(g * 768, 6) for g in range(65)] + [(49920, 3)]
                for gi, (r0, tpg) in enumerate(groups):
                    xt = pa.tile([128, 6 * 128], f32, tag="xt")
                    nc.sync.dma_start(
                        xt[:, :tpg * 128], xT[:, r0:r0 + tpg * 128])
                    hps = pap.tile([128, 6 * EXT], f32, space="PSUM",
                                   tag="hps")
                    for i in range(tpg):
                        nc.tensor.matmul(
                            hps[:, i * EXT:(i + 1) * EXT],
                            lhsT=xt[:, i * 128:(i + 1) * 128],
                            rhs=w_ext[:], start=True, stop=True)
                    hsb = pa.tile([128, 6, EXT], f32, tag="hsb")
                    if gi % 2 == 0:
                        nc.vector.tensor_copy(
                            hsb[:, :tpg].rearrange("p t c -> p (t c)"),
                            hps[:, :tpg * EXT])
                    else:
                        nc.scalar.copy(
                            hsb[:, :tpg].rearrange("p t c -> p (t c)"),
                            hps[:, :tpg * EXT])
                    nc.sync.dma_start(
                        htab[r0:r0 + tpg * 128, 0:EXT]
                        .rearrange("(t p) c -> p t c", p=128),
                        hsb[:, :tpg])
                    if r0 + tpg * 128 <= 6912:
                        nc.sync.dma_start(
                            htab_own[r0:r0 + tpg * 128, 0:EXT]
                            .rearrange("(t p) c -> p t c", p=128),
                            hsb[:, :tpg])

            # ---- Phase B: edge aggregation ------------------------------
            htabA = htab[0:SPLIT, 0:128]
            htabB = htab[SPLIT:HTAB_ROWS, 0:128]
            htabD = htab_own[0:NLOC, 64:128]
            with (tc.tile_pool(name="pb", bufs=3) as pb,
                  tc.tile_pool(name="pb_gd", bufs=8) as pbgd,
                  tc.tile_pool(name="pb_oh", bufs=28) as pboh,
                  tc.tile_pool(name="pb_ps", bufs=4, space="PSUM") as pbp):
                for w in range(NW):
                    gs = pb.tile([128, NB, 128], f32, tag="gs")
                    gd = pbgd.tile([128, NB, 64], f32, tag="gd")
                    col0 = w * NB * 8
                    qn = 0
                    for (b0, b1) in A_calls:
                        n = (b1 - b0) * 128
                        nc.gpsimd.dma_gather(
                            gs[:, b0:b1], htabA,
                            srcidx_sb[:, col0 + b0 * 8: col0 + b1 * 8],
                            num_idxs=n, num_idxs_reg=n, elem_size=128,
                            queue_num=qn % 4)
                        qn += 1
                    for (b0, b1) in B_calls:
                        n = (b1 - b0) * 128
                        nc.gpsimd.dma_gather(
                            gs[:, b0:b1], htabB,
                            srcidx_sb[:, col0 + b0 * 8: col0 + b1 * 8],
                            num_idxs=n, num_idxs_reg=n, elem_size=128,
                            queue_num=qn % 4)
                        qn += 1
                    for (b0, b1) in D_calls:
                        n = (b1 - b0) * 128
                        nc.gpsimd.dma_gather(
                            gd[:, b0:b1], htabD,
                            dstidx_sb[:, col0 + b0 * 8: col0 + b1 * 8],
                            num_idxs=n, num_idxs_reg=n, elem_size=64,
                            elem_step=128, queue_num=qn % 4)
                        qn += 1
                    et = pb.tile([128, NB, H], f32, tag="et")
                    nc.vector.tensor_tensor(
                        out=et[:], in0=gs[:, :, 64:68], in1=gd[:, :, 4:8],
                        op=bass.mybir.AluOpType.add)
                    # leaky_relu(z) = max(z, 0.2*z), exact on DVE
                    et2 = pb.tile([128, NB, H], f32, tag="et2")
                    nc.vector.tensor_scalar_mul(et2[:], et[:], NEG_SLOPE)
                    nc.vector.tensor_tensor(
                        out=et[:], in0=et[:], in1=et2[:],
                        op=bass.mybir.AluOpType.max)
                    nc.scalar.activation(
                        gs[:, :, 64:68], et[:],
                        bass.mybir.ActivationFunctionType.Exp)
                    # V = p (x) h, in place on gs cols 0:64
                    pexp = gs[:, :, 64:68]
                    p_bc = bass.AP(pexp.tensor, pexp.offset,
                                   [pexp.ap[0], pexp.ap[1], pexp.ap[2],
                                    [0, C]])
                    nc.vector.tensor_tensor(
                        out=gs[:, :, 0:64].rearrange("p b (h c) -> p b h c",
                                                     c=C),
                        in0=gs[:, :, 0:64].rearrange("p b (h c) -> p b h c",
                                                     c=C),
                        in1=p_bc, op=bass.mybir.AluOpType.mult)
                    ps = pbp.tile([128, 68], f32, space="PSUM", tag="pw")
                    for b in range(NB):
                        oh = pboh.tile([128, 128], f32, tag="oh")
                        nc.vector.tensor_scalar(
                            out=oh[:], in0=iota_f[:],
                            scalar1=wrel_sb[:, w * NB + b: w * NB + b + 1],
                            scalar2=None,
                            op0=bass.mybir.AluOpType.is_equal)
                        nc.tensor.matmul(ps[:], lhsT=oh[:],
                                         rhs=gs[:, b, 0:68],
                                         start=(b == 0), stop=(b == NB - 1))
                    nc.scalar.copy(sloc[:, w, :], ps[:])

            # ---- Phase C: normalize + bias ------------------------------
            with tc.tile_pool(name="pc", bufs=2) as pc:
                for w in range(NW):
                    dt = pc.tile([128, H], f32, tag="dt")
                    nc.vector.tensor_scalar_add(dt[:], sloc[:, w, 64:68],
                                                1e-16)
                    rc = pc.tile([128, H], f32, tag="rc")
                    nc.vector.reciprocal(rc[:], dt[:])
                    rbc = bass.AP(rc[:].tensor, rc[:].offset,
                                  [rc[:].ap[0], rc[:].ap[1], [0, C]])
                    ot = pc.tile([128, HC], f32, tag="ot")
                    nc.vector.tensor_tensor(
                        out=ot[:].rearrange("p (h c) -> p h c", c=C),
                        in0=sloc[:, w, 0:64].rearrange("p (h c) -> p h c",
                                                       c=C),
                        in1=rbc, op=bass.mybir.AluOpType.mult)
                    nc.vector.tensor_tensor(out=ot[:], in0=ot[:],
                                            in1=bias_bc[:],
                                            op=bass.mybir.AluOpType.add)
                    nc.sync.dma_start(out_t[w * 128:(w + 1) * 128, :], ot[:])

    nc.compile()
    return nc


def kernel(x, edge_index, W, att_src, att_dst, bias):
    from concourse.bass_utils import run_bass_kernel_spmd

    x = np.asarray(x)
    edge_index = np.asarray(edge_index)
    W = np.asarray(W)
    att_src = np.asarray(att_src)
    att_dst = np.asarray(att_dst)
    bias = np.asarray(bias)

    cores, capA, capB = _build_host_data(edge_index)
    NB = capA + capB

    key = (capA, capB)
    if key not in _cache:
        _cache[key] = _build_bass(capA, capB)
    nc = _cache[key]

    # host-side layout prep (no model math: att_blk places att values into a
    # block-diagonal zero matrix; a_src/a_dst projections are computed on
    # device as W @ att_blk)
    attblk = np.zeros((HC, 2 * H), np.float32)
    for h in range(H):
        attblk[h * C:(h + 1) * C, h] = att_src[h]
        attblk[h * C:(h + 1) * C, H + h] = att_dst[h]

    xT_full = np.zeros((IN, HTAB_ROWS), np.float32)
    xT_full[:, :N] = x.T

    A_calls = _chunks(0, capA)
    B_calls = _chunks(capA, NB)
    D_calls = _chunks(0, NB)

    in_maps = []
    perms = []
    for c in range(NCORES):
        srcidx, dstidx, wrelf, pos = cores[c]
        perms.append(pos)
        xT_c = np.roll(xT_full[:, :NPAD], -c * NLOC, axis=1)
        xT_c = np.concatenate(
            [xT_c, np.zeros((IN, HTAB_ROWS - NPAD), np.float32)], axis=1)
        in_maps.append({
            "xT": np.ascontiguousarray(xT_c),
            "W": W.astype(np.float32),
            "attblk": attblk,
            "bias": bias.astype(np.float32).reshape(1, HC),
            "srcidx": _wrap_calls(srcidx, A_calls + B_calls),
            "dstidx": _wrap_calls(dstidx, D_calls),
            "wrel": np.ascontiguousarray(
                wrelf.transpose(2, 0, 1).reshape(128, NW * NB)),
        })

    res = run_bass_kernel_spmd(nc, in_maps, core_ids=list(range(NCORES)))
    global LAST_RESULTS
    LAST_RESULTS = res
    out = np.concatenate(
        [res.results[c]["out"][perms[c]] for c in range(NCORES)], axis=0)
    return np.ascontiguousarray(out[:N])


LAST_RESULTS = None
